# revision 51
# baseline (speedup 1.0000x reference)
"""Causal self-attention kernel for 8 trn2 NeuronCores.

Sharding: core c handles batch b = c // 4 and local head group hg = c % 4
(4 of the 16 heads). Tensor-parallel over heads for kqv / attention and
row-parallel for the output projection; the 4 per-batch partial projections
are summed on the host (the "all-reduce" of classic TP), where the bias is
also added.

Inputs are pre-tiled on the host into [128, *] SBUF-ready layouts (128-row
panels packed along the free dim) so every load is a single large DMA.

Device kernel (per core, bf16 matmuls, fp32 accumulation). The attention
phase is paced by max(PE, ScalarE-exp), so all dense matmul work (next
window's kq/v projections, previous window's output projection) is emitted
as "filler units" interleaved into the attention j-loops.

Round-1 scheduling fixes vs the original baseline:
  - HAM warmup: dummy matmuls on a memset tile keep the PE busy from engine
    boot until the first input DMA lands, so real work runs at 2.4 GHz.
  - Input DMAs batched (9 issues instead of 15) - DMA_DIRECT2D issue costs
    ~600ns each on the Sync queue and was pacing the first window.
  - The ones blocks (softmax denominator trick) are memset on GpSimd with a
    strided AP (only the 64-col ones halves), off Vector's critical path.
  - reciprocal reads the denominator directly from PSUM (drops 16 ScalarE
    copies ~= 15us of ScalarE time).
  - proj(3) is split into its two head-pair halves: the oc_s[0] half runs
    as filler inside attn(3)'s second head-pair loop, the oc_s[1] half
    accumulates into SBUF right after the last normalize - the tail stays
    warm and short.
  - y output DMAs for windows 0-2 are batched in pairs.

Per tq window g of 512:
  kq(g):   kqT window = (Wqk x^T)[:, g]      [512 feat, 512 t]  k,q head-major
  v(g):    v chunks 4g..4g+3 = (x Wv^T)      [128 t, 4*(64 v | 64 ones)]
           (the ones blocks compute softmax denominators on the PE)
  attn(g): head pairs share a 1024-wide psum strip (h_even cols 0:512,
           h_odd cols 512:1024), tk chunks j <= 4g+3, causally trimmed:
             S^T = k^T.T q^T     (pair runs on PE row groups 0-63 / 64-127)
             P = exp(S^T/8)      one ScalarE activation per strip, no max
                                 subtraction (scores are O(1) by construction)
             causal mask         affine_select on GpSimd (diagonal squares)
             O^T psum[0:64]  += v_j.T  @ P
             psum[64:128]    += ones.T @ P   (denominator, replicated)
           normalize: reciprocal_approx_fast from PSUM, multiply (VectorE)
  proj(g): y[:, window] = O_cat^T.T @ Wp^T -> fp32 -> DMA (as filler in g+1)
"""

import numpy as np
import ml_dtypes

T = 2048
C = 1024
NH_LOCAL = 4
D = 64
TQW = 512  # tq window width
NGRP = T // TQW  # 4 tq windows

_nc_cache = {}


def _build_bass():
    import concourse.mybir as mybir
    import concourse.tile as tile
    from concourse import bacc

    f32 = mybir.dt.float32
    bf16 = mybir.dt.bfloat16
    f8 = mybir.dt.float8e4

    nc = bacc.Bacc(None, target_bir_lowering=False)
    # pre-tiled inputs: [128, packed free dim] (see _shard_inputs)
    # bf16 copies feed window 0 (short-context rows need the precision);
    # fp8 pair-interleaved copies feed windows 1-3 via DoubleRow matmuls
    xt_d = nc.dram_tensor("xt", [128, 4096], bf16, kind="ExternalInput")
    xtf_d = nc.dram_tensor("xtf", [128, 3 * 4096], f8, kind="ExternalInput")
    wqk_d = nc.dram_tensor("wqk", [128, 8 * 512], bf16, kind="ExternalInput")
    wqkf_d = nc.dram_tensor("wqkf", [128, 4096], f8, kind="ExternalInput")
    wv_d = nc.dram_tensor("wv", [128, 8 * 256], bf16, kind="ExternalInput")
    wvf_d = nc.dram_tensor("wvf", [128, 2048], f8, kind="ExternalInput")
    wp_d = nc.dram_tensor("wp", [128, 2 * C], bf16, kind="ExternalInput")
    y_d = nc.dram_tensor("y", [T, C], f32, kind="ExternalOutput")

    with tile.TileContext(nc) as tc:
        with (
            tc.tile_pool(name="persist", bufs=1) as pp,
            tc.tile_pool(name="mmp", bufs=2, space="PSUM") as mp,
            tc.tile_pool(name="spsum", bufs=2, space="PSUM") as sp,
            tc.tile_pool(name="opsum", bufs=1, space="PSUM") as op,
            tc.tile_pool(name="ptp", bufs=4) as ptp,
            tc.tile_pool(name="rp", bufs=4) as rp,
            tc.tile_pool(name="ysb", bufs=2) as ysb,
        ):
            xt_s = pp.tile([128, 4096], bf16, tag="xt", name="xt")
            xtf_s = pp.tile([128, 3 * 4096], f8, tag="xtf", name="xtf")
            wqk_s = pp.tile([128, 8 * 512], bf16, tag="wqk", name="wqk")
            wqkf_s = pp.tile([128, 4096], f8, tag="wqkf", name="wqkf")
            wv_s = pp.tile([128, 8 * 256], bf16, tag="wv", name="wv")
            wvf_s = pp.tile([128, 2048], f8, tag="wvf", name="wvf")
            wp_s = pp.tile([128, 2 * C], bf16, tag="wp", name="wp")
            kq_s = [pp.tile([128, T], bf16, tag=f"kq{f}", name=f"kq{f}") for f in range(4)]
            v_s = pp.tile([128, 4 * T], f8, tag="vall", name="vall")
            oc_s = [pp.tile([128, T], bf16, tag=f"oc{p}", name=f"oc{p}") for p in range(2)]
            wu_s = pp.tile([128, 512], bf16, tag="wu", name="wu")
            # bf16 copy of tk chunks j=0,1 (v | ones): short-context queries
            # (t<256) see little averaging, too noisy for the fp8 path
            vb_s = pp.tile([128, 1024], bf16, tag="vb", name="vb")
            # window-3 proj strips held in SBUF across the hp=0/hp=1 split
            ys3_s = [
                pp.tile([128, C], f32, tag=f"ys3_{i}", name=f"ys3_{i}")
                for i in range(4)
            ]

            # slicing helpers for the packed layouts
            def xt_w(c):  # bf16 moving operand, window 0, contraction chunk c
                return xt_s[:, 512 * c : 512 * (c + 1)]

            def xt_j(c, j):  # bf16 stationary for v chunks 0-3
                o = 512 * c + 128 * (j % 4)
                return xt_s[:, o : o + 128]

            def xtf_w(cp, g):  # fp8 [p,2,512] rhs pair view, windows 1-3
                o = 4096 * (g - 1) + 1024 * cp
                return xtf_s[:, o : o + 1024].rearrange(
                    "p (two x) -> p two x", two=2
                )

            def xtf_j(cp, j):  # fp8 [p,2,128] stationary pair view, chunks 4-15
                o = 4096 * (j // 4 - 1) + 1024 * cp
                t0 = 128 * (j % 4)
                return xtf_s[:, o : o + 1024].rearrange(
                    "p (two x) -> p two x", two=2
                )[:, :, t0 : t0 + 128]

            # ---- HAM warmup: PE busy from boot so real matmuls run warm ----
            nc.gpsimd.memset(wu_s[:], 0.03125)
            for _ in range(13):
                wacc = mp.tile([128, 512], f32, tag="mm", name="warm")
                nc.tensor.matmul(wacc[:], wu_s[:, 0:128], wu_s[:], start=True, stop=True)

            # ones blocks for the denominator trick (v regions overwritten
            # later); contiguous memset on GpSimd - off Vector's critical path
            nc.gpsimd.memset(v_s[:], 1.0)
            nc.gpsimd.memset(vb_s[:], 1.0)

            # input DMAs: first window's operands first, few large issues
            nc.sync.dma_start(wqk_s[:, 0:1024], wqk_d[:, 0:1024])
            nc.sync.dma_start(xt_s[:, 0:2048], xt_d[:, 0:2048])
            nc.sync.dma_start(xt_s[:, 2048:4096], xt_d[:, 2048:4096])
            nc.sync.dma_start(wqk_s[:, 1024:4096], wqk_d[:, 1024:4096])
            nc.sync.dma_start(wv_s[:], wv_d[:])
            nc.sync.dma_start(wqkf_s[:], wqkf_d[:])
            nc.sync.dma_start(xtf_s[:, 0:4096], xtf_d[:, 0:4096])
            nc.sync.dma_start(wvf_s[:], wvf_d[:])
            nc.sync.dma_start(wp_s[:], wp_d[:])
            nc.sync.dma_start(xtf_s[:, 4096:12288], xtf_d[:, 4096:12288])

            # ---- dense work generators (emitted one instruction at a time) ----
            def kq_units(g, fs=(0, 1, 2, 3)):
                units = []
                for f in fs:
                    st = {}
                    if g == 0:
                        for c in range(8):
                            def mm(f=f, c=c, st=st):
                                if c == 0:
                                    st["acc"] = mp.tile([128, 512], f32, tag="mm", name="mmkq")
                                nc.tensor.matmul(
                                    st["acc"][:],
                                    wqk_s[:, 1024 * f + 128 * c : 1024 * f + 128 * (c + 1)],
                                    xt_w(c),
                                    start=(c == 0),
                                    stop=(c == 7),
                                )
                            units.append(mm)
                    else:
                        for cp_ in range(4):
                            def mm(f=f, cp_=cp_, g=g, st=st):
                                if cp_ == 0:
                                    st["acc"] = mp.tile([128, 512], f32, tag="mm", name="mmkq")
                                nc.tensor.matmul(
                                    st["acc"][:],
                                    wqkf_s[
                                        :, 1024 * f + 256 * cp_ : 1024 * f + 256 * (cp_ + 1)
                                    ].rearrange("p (two x) -> p two x", two=2),
                                    xtf_w(cp_, g),
                                    start=(cp_ == 0),
                                    stop=(cp_ == 3),
                                    perf_mode=mybir.MatmulPerfMode.DoubleRow,
                                )
                            units.append(mm)
                    def cp(f=f, g=g, st=st):
                        nc.vector.tensor_copy(
                            kq_s[f][:, TQW * g : TQW * (g + 1)], st["acc"][:]
                        )
                    units.append(cp)
                return units

            def v_units(g, js=None):
                units = []
                for j in (js if js is not None else range(4 * g, 4 * g + 4)):
                    st = {}
                    if g == 0:
                        for c in range(8):
                            def mm(j=j, c=c, st=st):
                                if c == 0:
                                    st["acc"] = mp.tile([128, 512], f32, tag="mm", name="mmv")
                                nc.tensor.matmul(
                                    st["acc"][:, :256],
                                    xt_j(c, j),
                                    wv_s[:, 256 * c : 256 * (c + 1)],
                                    start=(c == 0),
                                    stop=(c == 7),
                                )
                            units.append(mm)
                    else:
                        for cp_ in range(4):
                            def mm(j=j, cp_=cp_, st=st):
                                if cp_ == 0:
                                    st["acc"] = mp.tile([128, 512], f32, tag="mm", name="mmv")
                                nc.tensor.matmul(
                                    st["acc"][:, :256],
                                    xtf_j(cp_, j),
                                    wvf_s[:, 512 * cp_ : 512 * (cp_ + 1)].rearrange(
                                        "p (two x) -> p two x", two=2
                                    ),
                                    start=(cp_ == 0),
                                    stop=(cp_ == 3),
                                    perf_mode=mybir.MatmulPerfMode.DoubleRow,
                                )
                            units.append(mm)
                    def cp(j=j, st=st):
                        nc.vector.tensor_copy(
                            v_s[:, 512 * j : 512 * j + 512].rearrange(
                                "p (h x) -> p h x", h=4
                            )[:, :, 0:64],
                            st["acc"][:, 0:256].rearrange("p (h x) -> p h x", h=4),
                        )
                        if j < 2:
                            nc.vector.tensor_copy(
                                vb_s[:, 512 * j : 512 * (j + 1)].rearrange(
                                    "p (h x) -> p h x", h=4
                                )[:, :, 0:64],
                                st["acc"][:, 0:256].rearrange("p (h x) -> p h x", h=4),
                            )
                    units.append(cp)
                return units

            def proj_units(g):
                # windows 0..2: full projection, y strips DMA'd in pairs
                units = []
                for pi in range(2):
                    i0 = 4 * g + 2 * pi
                    st = {}
                    def alloc(st=st):
                        st["ys"] = ysb.tile([128, 2 * C], f32, tag="ys", name="ys")
                    units.append(alloc)
                    for k in range(2):
                        i = i0 + k
                        for u in range(2):
                            for ci in range(2):
                                def mm(i=i, u=u, ci=ci, st=st):
                                    if ci == 0:
                                        st["acc"] = mp.tile([128, 512], f32, tag="mm", name="mmy")
                                    nc.tensor.matmul(
                                        st["acc"][:],
                                        oc_s[ci][:, 128 * i : 128 * (i + 1)],
                                        wp_s[:, 1024 * ci + 512 * u : 1024 * ci + 512 * (u + 1)],
                                        start=(ci == 0),
                                        stop=(ci == 1),
                                    )
                                units.append(mm)
                            def cp(k=k, u=u, st=st):
                                nc.vector.tensor_copy(
                                    st["ys"][:, 1024 * k + 512 * u : 1024 * k + 512 * (u + 1)],
                                    st["acc"][:],
                                )
                            units.append(cp)
                    def out(i0=i0, st=st):
                        nc.sync.dma_start(
                            y_d[128 * i0 : 128 * i0 + 256, :].rearrange(
                                "(two p) c -> p two c", two=2
                            ),
                            st["ys"][:].rearrange("p (two c) -> p two c", two=2),
                        )
                    units.append(out)
                return units

            def proj3_a_units():
                # window 3, oc_s[0] (head pair 0) half: filler for attn(3)
                # hp=1; partial y strips stream out early (bypass DMA)
                units = []
                for k, i in enumerate(range(12, 16)):
                    st = {}
                    for u in range(2):
                        def mm(i=i, u=u, st=st):
                            st["acc"] = mp.tile([128, 512], f32, tag="mm", name="mmy")
                            nc.tensor.matmul(
                                st["acc"][:],
                                oc_s[0][:, 128 * i : 128 * (i + 1)],
                                wp_s[:, 512 * u : 512 * (u + 1)],
                                start=True,
                                stop=True,
                            )
                        units.append(mm)
                        def cp(k=k, u=u, st=st):
                            nc.vector.tensor_copy(
                                ys3_s[k][:, 512 * u : 512 * (u + 1)], st["acc"][:]
                            )
                        units.append(cp)
                return units

            def proj3_b_units():
                # window 3, oc_s[1] half: accumulate into SBUF, DMA out
                units = []
                for k, i in enumerate(range(12, 16)):
                    st = {}
                    for u in range(2):
                        def mm(i=i, u=u, st=st):
                            st["acc"] = mp.tile([128, 512], f32, tag="mm", name="mmy")
                            nc.tensor.matmul(
                                st["acc"][:],
                                oc_s[1][:, 128 * i : 128 * (i + 1)],
                                wp_s[:, 1024 + 512 * u : 1024 + 512 * (u + 1)],
                                start=True,
                                stop=True,
                            )
                        units.append(mm)
                        def add(k=k, u=u, st=st):
                            nc.vector.tensor_tensor(
                                ys3_s[k][:, 512 * u : 512 * (u + 1)],
                                ys3_s[k][:, 512 * u : 512 * (u + 1)],
                                st["acc"][:],
                                mybir.AluOpType.add,
                            )
                        units.append(add)
                    def out(k=k, i=i):
                        nc.sync.dma_start(y_d[128 * i : 128 * (i + 1), :], ys3_s[k][:])
                    units.append(out)
                return units

            def drain(units):
                for u in units:
                    u()
                units.clear()

            def attn(g, filler, late_filler=None, boundary=None):
                w0 = TQW * g
                jmax = 4 * g + 3
                npairs = (jmax + 1) // 2
                nblocks_left = 2 * 2 * npairs
                for hp in range(2):
                    h0, h1 = 2 * hp, 2 * hp + 1
                    o_t = {
                        h0: op.tile([128, TQW], f32, tag="oh0", name="oh0"),
                        h1: op.tile([128, TQW], f32, tag="oh1", name="oh1"),
                    }
                    for m in range(npairs):
                        # window 0's first pair covers short-context queries
                        # (t<128) where fp8 noise does not average out - bf16
                        lowp = not (g == 0 and m == 0)
                        # pt2 layout: [j0: h0 512 | h1 512][j1: h0 512 | h1 512]
                        if lowp:
                            pt2 = ptp.tile([128, 4 * TQW], f8, tag="pt", name="pt")
                        else:
                            pt2 = ptp.tile([128, 4 * TQW], bf16, tag="ptb", name="ptb")
                        cs0 = max(0, 128 * (2 * m) - w0)
                        for p in range(2):
                            j = 2 * m + p
                            cs = max(0, 128 * j - w0)
                            s_t = sp.tile([128, 2 * TQW], f32, tag="s", name="s")
                            for idx, h in enumerate((h0, h1)):
                                # idx1 writes its full strip on diagonal blocks
                                # so one exp covers [cs:1024] without reading
                                # unwritten psum (extra cols are masked later)
                                ics = cs if idx == 0 else 0
                                kT = kq_s[h // 2][64 * (h % 2) : 64 * (h % 2) + 64, :]
                                qT = kq_s[2 + h // 2][64 * (h % 2) : 64 * (h % 2) + 64, :]
                                nc.tensor.matmul(
                                    s_t[:, 512 * idx + ics : 512 * idx + 512],
                                    kT[:, 128 * j : 128 * (j + 1)],
                                    qT[:, w0 + ics : w0 + TQW],
                                    start=True,
                                    stop=True,
                                )
                            po = 1024 * p
                            nc.scalar.activation(
                                pt2[:, po + cs : po + 2 * TQW],
                                s_t[:, cs : 2 * TQW],
                                mybir.ActivationFunctionType.Exp,
                                scale=float(D) ** -0.5,
                            )
                            # filler: dense matmuls the PE runs while exp
                            # cooks; hold back a few units to bridge the
                            # end-of-window normalize chain
                            avail = len(filler) - 6
                            npop = min(5, -(-avail // max(1, nblocks_left))) if avail > 0 else 0
                            for _ in range(min(npop, len(filler))):
                                filler.pop(0)()
                            nblocks_left -= 1
                            if 128 * j >= w0:
                                if p == 1 and cs > cs0:
                                    # zero j1's [cs0:cs) gap (tq rows that may
                                    # attend j0's block but not j1's)
                                    for idx in range(2):
                                        nc.gpsimd.memset(
                                            pt2[:, po + 512 * idx + cs0 : po + 512 * idx + cs],
                                            0.0,
                                        )
                                for idx in range(2):
                                    nc.gpsimd.affine_select(
                                        out=pt2[:, po + 512 * idx + cs : po + 512 * idx + cs + 128],
                                        in_=pt2[:, po + 512 * idx + cs : po + 512 * idx + cs + 128],
                                        compare_op=mybir.AluOpType.is_ge,
                                        fill=0.0,
                                        base=0,
                                        pattern=[[1, 128]],
                                        channel_multiplier=-1,
                                    )
                        if lowp:
                            # one fp8 DoubleRow matmul per head accumulates both
                            # tk chunks of the pair: lhsT/rhs are [K, 2, dim] views
                            for idx, h in enumerate((h0, h1)):
                                vv = v_s[:].rearrange("p (jj x) -> p jj x", jj=16)[
                                    :, 2 * m : 2 * m + 2, 128 * h : 128 * (h + 1)
                                ]
                                rr = pt2[:].rearrange("p (jj x) -> p jj x", jj=2)[
                                    :, :, 512 * idx + cs0 : 512 * (idx + 1)
                                ]
                                nc.tensor.matmul(
                                    o_t[h][:, cs0:TQW],
                                    vv,
                                    rr,
                                    start=(m == 0),
                                    stop=(m == npairs - 1),
                                    perf_mode=mybir.MatmulPerfMode.DoubleRow,
                                )
                        else:
                            # bf16: one matmul per (j, head)
                            for p in range(2):
                                jj = 2 * m + p
                                jcs = max(0, 128 * jj - w0)
                                for idx, h in enumerate((h0, h1)):
                                    nc.tensor.matmul(
                                        o_t[h][:, jcs:TQW],
                                        vb_s[:, 512 * p + 128 * h : 512 * p + 128 * (h + 1)],
                                        pt2[:, 1024 * p + 512 * idx + jcs : 1024 * p + 512 * (idx + 1)],
                                        start=(jj == 0),
                                        stop=False,
                                    )
                    # interleave the two heads' normalize chains so the
                    # Scalar-copy / DVE-recip / DVE-mult stages pipeline
                    lsb, rinv = {}, {}
                    for h in (h0, h1):
                        lsb[h] = rp.tile([64, 512], f32, tag="lsb", name="lsb")
                        # DVE copy keeps ScalarE free to pace the next exps
                        nc.vector.tensor_copy(lsb[h][:], o_t[h][64:128, :])
                    for h in (h0, h1):
                        rinv[h] = rp.tile([64, 512], f32, tag="rinv", name="rinv")
                        nc.vector.reciprocal_approx_fast(rinv[h][:], lsb[h][:])
                    for h in (h0, h1):
                        nc.vector.tensor_tensor(
                            oc_s[h // 2][
                                64 * (h % 2) : 64 * (h % 2) + 64, w0 : w0 + TQW
                            ],
                            o_t[h][0:64, :],
                            rinv[h][:],
                            mybir.AluOpType.mult,
                        )
                    if hp == 0 and boundary:
                        # work hp=1 depends on (e.g. its k/q features) must be
                        # fully emitted before its first score
                        drain(boundary)
                    if hp == 0 and late_filler:
                        filler.extend(late_filler)
                        late_filler.clear()

            # ---- schedule ----
            drain(kq_units(0))
            drain(v_units(0))
            for g in range(NGRP):
                filler = []
                late = None
                boundary = None
                if g == 0:
                    filler += kq_units(1) + v_units(1)
                elif g == 1:
                    filler += kq_units(2) + v_units(2)
                elif g == 2:
                    # window 3's k chunks are only needed late in attn(3);
                    # its q/v plus the ready projections fill attn(2)
                    filler += kq_units(3, fs=(2, 3)) + v_units(3)
                    filler += proj_units(0) + proj_units(1)
                else:
                    filler += kq_units(3, fs=(0, 1)) + proj_units(2)
                    late = proj3_a_units()
                attn(g, filler, late, boundary)
                drain(filler)
            drain(proj3_b_units())

    nc.compile()
    return nc


def get_nc():
    if "nc" not in _nc_cache:
        _nc_cache["nc"] = _build_bass()
    return _nc_cache["nc"]


def _shard_inputs(x, W_kqv, W_proj):
    """Build the 8 per-core input maps: shard, transpose, cast to bf16 and
    pack 128-row panels along the free dim."""
    bf16 = ml_dtypes.bfloat16

    def pack(a):  # [128*k, n] -> [128, k*n], panel-major along free dim
        k = a.shape[0] // 128
        return np.ascontiguousarray(
            a.reshape(k, 128, a.shape[1]).transpose(1, 0, 2).reshape(128, -1)
        ).astype(bf16)

    f8 = ml_dtypes.float8_e4m3

    in_maps = []
    for core in range(8):
        b, hg = core // 4, core % 4
        heads = range(4 * hg, 4 * hg + 4)
        xt = x[b].T  # [C, T]
        # bf16 xt, window 0 only: [128, c*512 + t']
        xtw = xt.reshape(8, 128, 4, 512)  # [c, p, g, t']
        xtp = np.ascontiguousarray(xtw[:, :, 0].transpose(1, 0, 2).reshape(128, -1)).astype(bf16)
        # fp8 xt, windows 1-3, contraction pairs: [128, (g-1)*4096 + cp*1024 + par*512 + t']
        xtf = np.ascontiguousarray(
            xt.reshape(4, 2, 128, 4, 512)[:, :, :, 1:]  # [cp, par, p, g-1, t']
            .transpose(2, 3, 0, 1, 4)
            .reshape(128, -1)
        ).astype(f8)
        k_rows = [W_kqv[64 * h : 64 * (h + 1)] for h in heads]
        q_rows = [W_kqv[C + 64 * h : C + 64 * (h + 1)] for h in heads]
        v_rows = [W_kqv[2 * C + 64 * h : 2 * C + 64 * (h + 1)] for h in heads]
        wqk_cat = np.concatenate(k_rows + q_rows, 0)  # [512 feat, 1024 c]
        # f-major packing: [p, f*1024 + c*128 + fi]
        wqk = np.ascontiguousarray(
            wqk_cat.reshape(4, 128, 8, 128).transpose(3, 0, 2, 1).reshape(128, -1)
        ).astype(bf16)
        # fp8 pair packing: [p, f*1024 + cp*256 + par*128 + fi]
        wqkf = np.ascontiguousarray(
            wqk_cat.reshape(4, 128, 4, 2, 128).transpose(4, 0, 2, 3, 1).reshape(128, -1)
        ).astype(f8)
        wv_cat = np.concatenate(v_rows, 0).T  # [1024 c, 256]
        wv = pack(wv_cat)
        # fp8 pair packing: [p, cp*512 + par*256 + x]
        wvf = np.ascontiguousarray(
            wv_cat.reshape(4, 2, 128, 256).transpose(2, 0, 1, 3).reshape(128, -1)
        ).astype(f8)
        wp = pack(W_proj[:, 256 * hg : 256 * (hg + 1)].T)
        in_maps.append(
            {"xt": xtp, "xtf": xtf, "wqk": wqk, "wqkf": wqkf,
             "wv": wv, "wvf": wvf, "wp": wp}
        )
    return in_maps


def kernel(x, W_kqv, W_proj, b_proj):
    from concourse.bass_utils import run_bass_kernel_spmd

    x = np.asarray(x, dtype=np.float32)
    W_kqv = np.asarray(W_kqv, dtype=np.float32)
    W_proj = np.asarray(W_proj, dtype=np.float32)
    b_proj = np.asarray(b_proj, dtype=np.float32)
    nc = get_nc()
    in_maps = _shard_inputs(x, W_kqv, W_proj)
    res = run_bass_kernel_spmd(nc, in_maps, core_ids=list(range(8)))
    B = x.shape[0]
    out = np.empty((B, T, C), np.float32)
    for b in range(B):
        acc = res.results[4 * b]["y"].astype(np.float32).copy()
        for hg in range(1, 4):
            acc += res.results[4 * b + hg]["y"]
        out[b] = acc + b_proj[None, :]
    return out


# revision 53
# speedup vs baseline: 1.0147x; 1.0147x over previous
"""Causal self-attention kernel for 8 trn2 NeuronCores.

Sharding: core c handles batch b = c // 4 and local head group hg = c % 4
(4 of the 16 heads). Tensor-parallel over heads for kqv / attention and
row-parallel for the output projection; the 4 per-batch partial projections
are summed on the host (the "all-reduce" of classic TP), where the bias is
also added.

Inputs are pre-tiled on the host into [128, *] SBUF-ready layouts (128-row
panels packed along the free dim) so every load is a single large DMA.

Device kernel (per core; bf16 + fp8 DoubleRow matmuls, fp32 accumulation).
The attention phase is paced by max(PE, ScalarE-exp), so all dense matmul
work (later windows' k/q/v projections, earlier windows' output projection)
is emitted as "filler units" interleaved into the attention loops, sized to
keep the PE both busy and HAM-warm (2.4 GHz) end to end.

Key mechanisms:
  - HAM warmup: dummy matmuls on a memset tile cover engine boot + the
    input-DMA latency so real work starts at full clock.
  - fp8e4 DoubleRow everywhere precision allows: the attention O-matmuls
    process tk chunks in PAIRS (lhsT/rhs are [K,2,dim] strided views over
    v and the exp strips), and the kq / v projections for windows 1-3
    contract channel pairs from fp8 copies of x / W. Short-context queries
    (t<256) are served by a bf16 path (first j-pair of window 0, bf16
    chunks in vb_s) because fp8 noise does not average out there.
  - exp strips write a [j0: h0|h1 | j1: h0|h1] fp8 tile; on diagonal
    blocks the activation is split per head so only matmul-written psum is
    read, and a GpSimd memset zeroes j1's causally-dead gap in each pair.
  - denominators via the ones-columns trick: the O-matmul stationary is
    (64 v | 64 ones), so psum rows 64:128 accumulate sum(P) for free.
  - proj(3) splits into head-pair halves: the oc_s[0] half fills attn(3)'s
    hp=1 loop, the oc_s[1] half adds into SBUF right after the last
    normalize, so the tail stays warm and short.
  - input DMAs are few large issues ordered by first use; y DMAs for
    windows 0-2 are batched in strip pairs.

Per tq window g of 512:
  kq(g):   kqT window = (Wqk x^T)[:, g]      [512 feat, 512 t]  k,q head-major
  v(g):    v chunks 4g..4g+3 = (x Wv^T)      [128 t, 4*(64 v | 64 ones)]
  attn(g): head pairs share 1024-wide psum strips (h_even 0:512, h_odd
           512:1024), tk chunk pairs m <= 2g+1, causally trimmed:
             S^T = k^T.T q^T     (head pair runs on PE row groups 0-63/64-127)
             P = exp(S^T/8)      fp8, no max subtraction (scores are O(1))
             causal mask         affine_select on GpSimd (diagonal squares)
             O^T psum[0:64]  += [v_2m v_2m+1] @ [P_2m P_2m+1]  (DoubleRow)
             psum[64:128]    += ones.T @ P   (denominator, replicated)
           normalize: DVE copy + reciprocal_approx_fast + multiply
  proj(g): y[:, window] = O_cat^T.T @ Wp^T -> fp32 -> DMA (as filler later)
"""

import numpy as np
import ml_dtypes

T = 2048
C = 1024
NH_LOCAL = 4
D = 64
TQW = 512  # tq window width
NGRP = T // TQW  # 4 tq windows

_nc_cache = {}


def _build_bass():
    import concourse.mybir as mybir
    import concourse.tile as tile
    from concourse import bacc

    f32 = mybir.dt.float32
    bf16 = mybir.dt.bfloat16
    f8 = mybir.dt.float8e4

    nc = bacc.Bacc(None, target_bir_lowering=False)
    # pre-tiled inputs: [128, packed free dim] (see _shard_inputs)
    # bf16 copies feed window 0 (short-context rows need the precision);
    # fp8 pair-interleaved copies feed windows 1-3 via DoubleRow matmuls
    xt_d = nc.dram_tensor("xt", [128, 4096], bf16, kind="ExternalInput")
    xtf_d = nc.dram_tensor("xtf", [128, 3 * 4096], f8, kind="ExternalInput")
    wqk_d = nc.dram_tensor("wqk", [128, 8 * 512], bf16, kind="ExternalInput")
    wqkf_d = nc.dram_tensor("wqkf", [128, 4096], f8, kind="ExternalInput")
    wv_d = nc.dram_tensor("wv", [128, 8 * 256], bf16, kind="ExternalInput")
    wvf_d = nc.dram_tensor("wvf", [128, 2048], f8, kind="ExternalInput")
    wp_d = nc.dram_tensor("wp", [128, 2 * C], bf16, kind="ExternalInput")
    y_d = nc.dram_tensor("y", [T, C], f32, kind="ExternalOutput")

    with tile.TileContext(nc) as tc:
        with (
            tc.tile_pool(name="persist", bufs=1) as pp,
            tc.tile_pool(name="mmp", bufs=2, space="PSUM") as mp,
            tc.tile_pool(name="spsum", bufs=2, space="PSUM") as sp,
            tc.tile_pool(name="opsum", bufs=1, space="PSUM") as op,
            tc.tile_pool(name="ptp", bufs=4) as ptp,
            tc.tile_pool(name="rp", bufs=4) as rp,
            tc.tile_pool(name="ysb", bufs=2) as ysb,
        ):
            xt_s = pp.tile([128, 4096], bf16, tag="xt", name="xt")
            xtf_s = pp.tile([128, 3 * 4096], f8, tag="xtf", name="xtf")
            wqk_s = pp.tile([128, 8 * 512], bf16, tag="wqk", name="wqk")
            wqkf_s = pp.tile([128, 4096], f8, tag="wqkf", name="wqkf")
            wv_s = pp.tile([128, 8 * 256], bf16, tag="wv", name="wv")
            wvf_s = pp.tile([128, 2048], f8, tag="wvf", name="wvf")
            wp_s = pp.tile([128, 2 * C], bf16, tag="wp", name="wp")
            kq_s = [pp.tile([128, T], bf16, tag=f"kq{f}", name=f"kq{f}") for f in range(4)]
            v_s = pp.tile([128, 4 * T], f8, tag="vall", name="vall")
            oc_s = [pp.tile([128, T], bf16, tag=f"oc{p}", name=f"oc{p}") for p in range(2)]
            wu_s = pp.tile([128, 512], bf16, tag="wu", name="wu")
            # bf16 copy of tk chunks j=0,1 (v | ones): short-context queries
            # (t<256) see little averaging, too noisy for the fp8 path
            vb_s = pp.tile([128, 1024], bf16, tag="vb", name="vb")
            # window-3 proj strips held in SBUF across the hp=0/hp=1 split
            ys3_s = [
                pp.tile([128, C], f32, tag=f"ys3_{i}", name=f"ys3_{i}")
                for i in range(4)
            ]

            # slicing helpers for the packed layouts
            def xt_w(c):  # bf16 moving operand, window 0, contraction chunk c
                return xt_s[:, 512 * c : 512 * (c + 1)]

            def xt_j(c, j):  # bf16 stationary for v chunks 0-3
                o = 512 * c + 128 * (j % 4)
                return xt_s[:, o : o + 128]

            def xtf_w(cp, g):  # fp8 [p,2,512] rhs pair view, windows 1-3
                o = 4096 * (g - 1) + 1024 * cp
                return xtf_s[:, o : o + 1024].rearrange(
                    "p (two x) -> p two x", two=2
                )

            def xtf_j(cp, j):  # fp8 [p,2,128] stationary pair view, chunks 4-15
                o = 4096 * (j // 4 - 1) + 1024 * cp
                t0 = 128 * (j % 4)
                return xtf_s[:, o : o + 1024].rearrange(
                    "p (two x) -> p two x", two=2
                )[:, :, t0 : t0 + 128]

            # ---- HAM warmup: PE busy from boot so real matmuls run warm ----
            nc.gpsimd.memset(wu_s[:], 0.03125)
            for _ in range(13):
                wacc = mp.tile([128, 512], f32, tag="mm", name="warm")
                nc.tensor.matmul(wacc[:], wu_s[:, 0:128], wu_s[:], start=True, stop=True)

            # ones blocks for the denominator trick (v regions overwritten
            # later); contiguous memset on GpSimd - off Vector's critical path
            nc.gpsimd.memset(v_s[:], 1.0)
            nc.gpsimd.memset(vb_s[:], 1.0)

            # input DMAs: first window's operands first, few large issues
            nc.sync.dma_start(wqk_s[:, 0:1024], wqk_d[:, 0:1024])
            nc.sync.dma_start(xt_s[:, 0:2048], xt_d[:, 0:2048])
            nc.sync.dma_start(xt_s[:, 2048:4096], xt_d[:, 2048:4096])
            nc.sync.dma_start(wqk_s[:, 1024:4096], wqk_d[:, 1024:4096])
            nc.sync.dma_start(wv_s[:], wv_d[:])
            nc.sync.dma_start(wqkf_s[:], wqkf_d[:])
            nc.sync.dma_start(xtf_s[:, 0:4096], xtf_d[:, 0:4096])
            nc.sync.dma_start(wvf_s[:], wvf_d[:])
            nc.sync.dma_start(wp_s[:], wp_d[:])
            nc.sync.dma_start(xtf_s[:, 4096:12288], xtf_d[:, 4096:12288])

            # ---- dense work generators (emitted one instruction at a time) ----
            def kq_units(g, fs=(0, 1, 2, 3)):
                units = []
                for f in fs:
                    st = {}
                    if g == 0:
                        for c in range(8):
                            def mm(f=f, c=c, st=st):
                                if c == 0:
                                    st["acc"] = mp.tile([128, 512], f32, tag="mm", name="mmkq")
                                nc.tensor.matmul(
                                    st["acc"][:],
                                    wqk_s[:, 1024 * f + 128 * c : 1024 * f + 128 * (c + 1)],
                                    xt_w(c),
                                    start=(c == 0),
                                    stop=(c == 7),
                                )
                            units.append(mm)
                    else:
                        for cp_ in range(4):
                            def mm(f=f, cp_=cp_, g=g, st=st):
                                if cp_ == 0:
                                    st["acc"] = mp.tile([128, 512], f32, tag="mm", name="mmkq")
                                nc.tensor.matmul(
                                    st["acc"][:],
                                    wqkf_s[
                                        :, 1024 * f + 256 * cp_ : 1024 * f + 256 * (cp_ + 1)
                                    ].rearrange("p (two x) -> p two x", two=2),
                                    xtf_w(cp_, g),
                                    start=(cp_ == 0),
                                    stop=(cp_ == 3),
                                    perf_mode=mybir.MatmulPerfMode.DoubleRow,
                                )
                            units.append(mm)
                    def cp(f=f, g=g, st=st):
                        nc.vector.tensor_copy(
                            kq_s[f][:, TQW * g : TQW * (g + 1)], st["acc"][:]
                        )
                    units.append(cp)
                return units

            def v_units(g, js=None):
                units = []
                for j in (js if js is not None else range(4 * g, 4 * g + 4)):
                    st = {}
                    if g == 0:
                        for c in range(8):
                            def mm(j=j, c=c, st=st):
                                if c == 0:
                                    st["acc"] = mp.tile([128, 512], f32, tag="mm", name="mmv")
                                nc.tensor.matmul(
                                    st["acc"][:, :256],
                                    xt_j(c, j),
                                    wv_s[:, 256 * c : 256 * (c + 1)],
                                    start=(c == 0),
                                    stop=(c == 7),
                                )
                            units.append(mm)
                    else:
                        for cp_ in range(4):
                            def mm(j=j, cp_=cp_, st=st):
                                if cp_ == 0:
                                    st["acc"] = mp.tile([128, 512], f32, tag="mm", name="mmv")
                                nc.tensor.matmul(
                                    st["acc"][:, :256],
                                    xtf_j(cp_, j),
                                    wvf_s[:, 512 * cp_ : 512 * (cp_ + 1)].rearrange(
                                        "p (two x) -> p two x", two=2
                                    ),
                                    start=(cp_ == 0),
                                    stop=(cp_ == 3),
                                    perf_mode=mybir.MatmulPerfMode.DoubleRow,
                                )
                            units.append(mm)
                    def cp(j=j, st=st):
                        nc.vector.tensor_copy(
                            v_s[:, 512 * j : 512 * j + 512].rearrange(
                                "p (h x) -> p h x", h=4
                            )[:, :, 0:64],
                            st["acc"][:, 0:256].rearrange("p (h x) -> p h x", h=4),
                        )
                        if j < 2:
                            nc.vector.tensor_copy(
                                vb_s[:, 512 * j : 512 * (j + 1)].rearrange(
                                    "p (h x) -> p h x", h=4
                                )[:, :, 0:64],
                                st["acc"][:, 0:256].rearrange("p (h x) -> p h x", h=4),
                            )
                    units.append(cp)
                return units

            def proj_units(g):
                # windows 0..2: full projection, y strips DMA'd in pairs
                units = []
                for pi in range(2):
                    i0 = 4 * g + 2 * pi
                    st = {}
                    def alloc(st=st):
                        st["ys"] = ysb.tile([128, 2 * C], f32, tag="ys", name="ys")
                    units.append(alloc)
                    for k in range(2):
                        i = i0 + k
                        for u in range(2):
                            for ci in range(2):
                                def mm(i=i, u=u, ci=ci, st=st):
                                    if ci == 0:
                                        st["acc"] = mp.tile([128, 512], f32, tag="mm", name="mmy")
                                    nc.tensor.matmul(
                                        st["acc"][:],
                                        oc_s[ci][:, 128 * i : 128 * (i + 1)],
                                        wp_s[:, 1024 * ci + 512 * u : 1024 * ci + 512 * (u + 1)],
                                        start=(ci == 0),
                                        stop=(ci == 1),
                                    )
                                units.append(mm)
                            def cp(k=k, u=u, st=st):
                                nc.vector.tensor_copy(
                                    st["ys"][:, 1024 * k + 512 * u : 1024 * k + 512 * (u + 1)],
                                    st["acc"][:],
                                )
                            units.append(cp)
                    def out(i0=i0, st=st):
                        nc.sync.dma_start(
                            y_d[128 * i0 : 128 * i0 + 256, :].rearrange(
                                "(two p) c -> p two c", two=2
                            ),
                            st["ys"][:].rearrange("p (two c) -> p two c", two=2),
                        )
                    units.append(out)
                return units

            def proj3_a_units():
                # window 3, oc_s[0] (head pair 0) half: filler for attn(3)
                # hp=1; partial y strips stream out early (bypass DMA)
                units = []
                for k, i in enumerate(range(12, 16)):
                    st = {}
                    for u in range(2):
                        def mm(i=i, u=u, st=st):
                            st["acc"] = mp.tile([128, 512], f32, tag="mm", name="mmy")
                            nc.tensor.matmul(
                                st["acc"][:],
                                oc_s[0][:, 128 * i : 128 * (i + 1)],
                                wp_s[:, 512 * u : 512 * (u + 1)],
                                start=True,
                                stop=True,
                            )
                        units.append(mm)
                        def cp(k=k, u=u, st=st):
                            nc.vector.tensor_copy(
                                ys3_s[k][:, 512 * u : 512 * (u + 1)], st["acc"][:]
                            )
                        units.append(cp)
                return units

            def proj3_b_units():
                # window 3, oc_s[1] half: accumulate into SBUF, DMA out
                units = []
                for k, i in enumerate(range(12, 16)):
                    st = {}
                    for u in range(2):
                        def mm(i=i, u=u, st=st):
                            st["acc"] = mp.tile([128, 512], f32, tag="mm", name="mmy")
                            nc.tensor.matmul(
                                st["acc"][:],
                                oc_s[1][:, 128 * i : 128 * (i + 1)],
                                wp_s[:, 1024 + 512 * u : 1024 + 512 * (u + 1)],
                                start=True,
                                stop=True,
                            )
                        units.append(mm)
                        def add(k=k, u=u, st=st):
                            nc.vector.tensor_tensor(
                                ys3_s[k][:, 512 * u : 512 * (u + 1)],
                                ys3_s[k][:, 512 * u : 512 * (u + 1)],
                                st["acc"][:],
                                mybir.AluOpType.add,
                            )
                        units.append(add)
                    def out(k=k, i=i):
                        nc.sync.dma_start(y_d[128 * i : 128 * (i + 1), :], ys3_s[k][:])
                    units.append(out)
                return units

            def drain(units):
                for u in units:
                    u()
                units.clear()

            def attn(g, filler, late_filler=None, boundary=None):
                w0 = TQW * g
                jmax = 4 * g + 3
                npairs = (jmax + 1) // 2
                nblocks_left = 2 * 2 * npairs
                for hp in range(2):
                    h0, h1 = 2 * hp, 2 * hp + 1
                    o_t = {
                        h0: op.tile([128, TQW], f32, tag="oh0", name="oh0"),
                        h1: op.tile([128, TQW], f32, tag="oh1", name="oh1"),
                    }
                    for m in range(npairs):
                        # window 0's first pair covers short-context queries
                        # (t<128) where fp8 noise does not average out - bf16
                        lowp = not (g == 0 and m == 0)
                        # pt2 layout: [j0: h0 512 | h1 512][j1: h0 512 | h1 512]
                        if lowp:
                            pt2 = ptp.tile([128, 4 * TQW], f8, tag="pt", name="pt")
                        else:
                            pt2 = ptp.tile([128, 4 * TQW], bf16, tag="ptb", name="ptb")
                        cs0 = max(0, 128 * (2 * m) - w0)
                        for p in range(2):
                            j = 2 * m + p
                            cs = max(0, 128 * j - w0)
                            s_t = sp.tile([128, 2 * TQW], f32, tag="s", name="s")
                            for idx, h in enumerate((h0, h1)):
                                kT = kq_s[h // 2][64 * (h % 2) : 64 * (h % 2) + 64, :]
                                qT = kq_s[2 + h // 2][64 * (h % 2) : 64 * (h % 2) + 64, :]
                                nc.tensor.matmul(
                                    s_t[:, 512 * idx + cs : 512 * idx + 512],
                                    kT[:, 128 * j : 128 * (j + 1)],
                                    qT[:, w0 + cs : w0 + TQW],
                                    start=True,
                                    stop=True,
                                )
                            po = 1024 * p
                            if cs:
                                # diagonal blocks: exp each half separately so
                                # only matmul-written psum bytes are read
                                for idx in range(2):
                                    nc.scalar.activation(
                                        pt2[:, po + 512 * idx + cs : po + 512 * (idx + 1)],
                                        s_t[:, 512 * idx + cs : 512 * idx + 512],
                                        mybir.ActivationFunctionType.Exp,
                                        scale=float(D) ** -0.5,
                                    )
                            else:
                                nc.scalar.activation(
                                    pt2[:, po : po + 2 * TQW],
                                    s_t[:, 0 : 2 * TQW],
                                    mybir.ActivationFunctionType.Exp,
                                    scale=float(D) ** -0.5,
                                )
                            # filler: dense matmuls the PE runs while exp
                            # cooks; hold back a few units to bridge the
                            # end-of-window normalize chain
                            avail = len(filler) - 6
                            npop = min(5, -(-avail // max(1, nblocks_left))) if avail > 0 else 0
                            for _ in range(min(npop, len(filler))):
                                filler.pop(0)()
                            nblocks_left -= 1
                            if 128 * j >= w0:
                                if p == 1 and cs > cs0:
                                    # zero j1's [cs0:cs) gap (tq rows that may
                                    # attend j0's block but not j1's)
                                    for idx in range(2):
                                        nc.gpsimd.memset(
                                            pt2[:, po + 512 * idx + cs0 : po + 512 * idx + cs],
                                            0.0,
                                        )
                                for idx in range(2):
                                    nc.gpsimd.affine_select(
                                        out=pt2[:, po + 512 * idx + cs : po + 512 * idx + cs + 128],
                                        in_=pt2[:, po + 512 * idx + cs : po + 512 * idx + cs + 128],
                                        compare_op=mybir.AluOpType.is_ge,
                                        fill=0.0,
                                        base=0,
                                        pattern=[[1, 128]],
                                        channel_multiplier=-1,
                                    )
                        if lowp:
                            # one fp8 DoubleRow matmul per head accumulates both
                            # tk chunks of the pair: lhsT/rhs are [K, 2, dim] views
                            for idx, h in enumerate((h0, h1)):
                                vv = v_s[:].rearrange("p (jj x) -> p jj x", jj=16)[
                                    :, 2 * m : 2 * m + 2, 128 * h : 128 * (h + 1)
                                ]
                                rr = pt2[:].rearrange("p (jj x) -> p jj x", jj=2)[
                                    :, :, 512 * idx + cs0 : 512 * (idx + 1)
                                ]
                                nc.tensor.matmul(
                                    o_t[h][:, cs0:TQW],
                                    vv,
                                    rr,
                                    start=(m == 0),
                                    stop=(m == npairs - 1),
                                    perf_mode=mybir.MatmulPerfMode.DoubleRow,
                                )
                        else:
                            # bf16: one matmul per (j, head)
                            for p in range(2):
                                jj = 2 * m + p
                                jcs = max(0, 128 * jj - w0)
                                for idx, h in enumerate((h0, h1)):
                                    nc.tensor.matmul(
                                        o_t[h][:, jcs:TQW],
                                        vb_s[:, 512 * p + 128 * h : 512 * p + 128 * (h + 1)],
                                        pt2[:, 1024 * p + 512 * idx + jcs : 1024 * p + 512 * (idx + 1)],
                                        start=(jj == 0),
                                        stop=False,
                                    )
                    # interleave the two heads' normalize chains so the
                    # Scalar-copy / DVE-recip / DVE-mult stages pipeline
                    lsb, rinv = {}, {}
                    for h in (h0, h1):
                        lsb[h] = rp.tile([64, 512], f32, tag="lsb", name="lsb")
                        # DVE copy keeps ScalarE free to pace the next exps
                        nc.vector.tensor_copy(lsb[h][:], o_t[h][64:128, :])
                    for h in (h0, h1):
                        rinv[h] = rp.tile([64, 512], f32, tag="rinv", name="rinv")
                        nc.vector.reciprocal_approx_fast(rinv[h][:], lsb[h][:])
                    for h in (h0, h1):
                        nc.vector.tensor_tensor(
                            oc_s[h // 2][
                                64 * (h % 2) : 64 * (h % 2) + 64, w0 : w0 + TQW
                            ],
                            o_t[h][0:64, :],
                            rinv[h][:],
                            mybir.AluOpType.mult,
                        )
                    if hp == 0 and boundary:
                        # work hp=1 depends on (e.g. its k/q features) must be
                        # fully emitted before its first score
                        drain(boundary)
                    if hp == 0 and late_filler:
                        filler.extend(late_filler)
                        late_filler.clear()

            # ---- schedule ----
            drain(kq_units(0))
            drain(v_units(0))
            for g in range(NGRP):
                filler = []
                late = None
                boundary = None
                if g == 0:
                    filler += kq_units(1) + v_units(1)
                elif g == 1:
                    filler += kq_units(2) + v_units(2)
                elif g == 2:
                    # window 3's k chunks are only needed late in attn(3);
                    # its q/v plus the ready projections fill attn(2)
                    filler += kq_units(3, fs=(2, 3)) + v_units(3)
                    filler += proj_units(0) + proj_units(1)
                else:
                    filler += kq_units(3, fs=(0, 1)) + proj_units(2)
                    late = proj3_a_units()
                attn(g, filler, late, boundary)
                drain(filler)
            drain(proj3_b_units())

    nc.compile()
    return nc


def get_nc():
    if "nc" not in _nc_cache:
        _nc_cache["nc"] = _build_bass()
    return _nc_cache["nc"]


def _shard_inputs(x, W_kqv, W_proj):
    """Build the 8 per-core input maps: shard, transpose, cast to bf16 and
    pack 128-row panels along the free dim."""
    bf16 = ml_dtypes.bfloat16

    def pack(a):  # [128*k, n] -> [128, k*n], panel-major along free dim
        k = a.shape[0] // 128
        return np.ascontiguousarray(
            a.reshape(k, 128, a.shape[1]).transpose(1, 0, 2).reshape(128, -1)
        ).astype(bf16)

    f8 = ml_dtypes.float8_e4m3

    in_maps = []
    for core in range(8):
        b, hg = core // 4, core % 4
        heads = range(4 * hg, 4 * hg + 4)
        xt = x[b].T  # [C, T]
        # bf16 xt, window 0 only: [128, c*512 + t']
        xtw = xt.reshape(8, 128, 4, 512)  # [c, p, g, t']
        xtp = np.ascontiguousarray(xtw[:, :, 0].transpose(1, 0, 2).reshape(128, -1)).astype(bf16)
        # fp8 xt, windows 1-3, contraction pairs: [128, (g-1)*4096 + cp*1024 + par*512 + t']
        xtf = np.ascontiguousarray(
            xt.reshape(4, 2, 128, 4, 512)[:, :, :, 1:]  # [cp, par, p, g-1, t']
            .transpose(2, 3, 0, 1, 4)
            .reshape(128, -1)
        ).astype(f8)
        k_rows = [W_kqv[64 * h : 64 * (h + 1)] for h in heads]
        q_rows = [W_kqv[C + 64 * h : C + 64 * (h + 1)] for h in heads]
        v_rows = [W_kqv[2 * C + 64 * h : 2 * C + 64 * (h + 1)] for h in heads]
        wqk_cat = np.concatenate(k_rows + q_rows, 0)  # [512 feat, 1024 c]
        # f-major packing: [p, f*1024 + c*128 + fi]
        wqk = np.ascontiguousarray(
            wqk_cat.reshape(4, 128, 8, 128).transpose(3, 0, 2, 1).reshape(128, -1)
        ).astype(bf16)
        # fp8 pair packing: [p, f*1024 + cp*256 + par*128 + fi]
        wqkf = np.ascontiguousarray(
            wqk_cat.reshape(4, 128, 4, 2, 128).transpose(4, 0, 2, 3, 1).reshape(128, -1)
        ).astype(f8)
        wv_cat = np.concatenate(v_rows, 0).T  # [1024 c, 256]
        wv = pack(wv_cat)
        # fp8 pair packing: [p, cp*512 + par*256 + x]
        wvf = np.ascontiguousarray(
            wv_cat.reshape(4, 2, 128, 256).transpose(2, 0, 1, 3).reshape(128, -1)
        ).astype(f8)
        wp = pack(W_proj[:, 256 * hg : 256 * (hg + 1)].T)
        in_maps.append(
            {"xt": xtp, "xtf": xtf, "wqk": wqk, "wqkf": wqkf,
             "wv": wv, "wvf": wvf, "wp": wp}
        )
    return in_maps


def kernel(x, W_kqv, W_proj, b_proj):
    from concourse.bass_utils import run_bass_kernel_spmd

    x = np.asarray(x, dtype=np.float32)
    W_kqv = np.asarray(W_kqv, dtype=np.float32)
    W_proj = np.asarray(W_proj, dtype=np.float32)
    b_proj = np.asarray(b_proj, dtype=np.float32)
    nc = get_nc()
    in_maps = _shard_inputs(x, W_kqv, W_proj)
    res = run_bass_kernel_spmd(nc, in_maps, core_ids=list(range(8)))
    B = x.shape[0]
    out = np.empty((B, T, C), np.float32)
    for b in range(B):
        acc = res.results[4 * b]["y"].astype(np.float32).copy()
        for hg in range(1, 4):
            acc += res.results[4 * b + hg]["y"]
        out[b] = acc + b_proj[None, :]
    return out
